# revision 5
# baseline (speedup 1.0000x reference)
"""Causal self-attention kernel v2 for Trainium2, data-parallel over batch.

Reference computation (B=256, T=256, C=192, H=6, D=32):
    qkv = x @ w_qkv.T -> q,k,v ; scores = q k^T / sqrt(D) causal-masked
    y = softmax(scores) @ v ; out = y @ w_out.T

v2 redesign vs v1 (PE-row + elementwise balance):
  - scores S^T[tk, tq] per head as before, but pair-tile column layout
    packs the 4 causal-diagonal 128-blocks at {0,128,512,640} so ONE
    affine_select per pair masks them (valid blocks at 256:512 skip it)
  - attn@v uses A^T blocks as the STATIONARY operand and V (+ ones col)
    as moving: Y[tq, 33] per (head, tq-chunk) -> 18 matmuls of N=33
    instead of 24 of N=128..256; col 32 accumulates the softmax row-sum
    for free (no separate ones-matmul broadcast pass)
  - normalize via reciprocal of the sum column + broadcast_to multiply
  - y -> y^T via PE transposes; out-proj from y^T stationary as before
  - qk^T quadrant shuffle merged into one DMA (4 fallback)
  - copies spread across Act (xt16), DVE (x16/qk16/v16/ydiv), Pool
    (yT16, out staging)
"""
import sys

sys.path.insert(0, "/opt/trn_rl_repo")

import numpy as np

B, T, C, H, D = 256, 256, 192, 6, 32
NCORES = 8
BPC = B // NCORES
SCALE = 1.0 / np.sqrt(np.float32(D))

# column offsets of each head's q^T / k^T block inside the [32, 3072] shuffle
QCOL = [0, 768, 1536, 2304, 256, 1024]
KCOL = [1792, 2560, 512, 1280, 2048, 2816]

_CACHE = {}


def _build(bpc=BPC, repeat=0, merged_shuffle=False):
    import contextlib
    from concourse import bacc, tile, mybir
    from concourse.masks import make_identity

    F32 = mybir.dt.float32
    F16 = mybir.dt.float16
    Exp = mybir.ActivationFunctionType.Exp

    nc = bacc.Bacc(None, target_bir_lowering=False)
    x_d = nc.dram_tensor("x", [bpc, T, C], F32, kind="ExternalInput")
    wqkv_d = nc.dram_tensor("w_qkv", [3 * C, C], F32, kind="ExternalInput")
    wout_d = nc.dram_tensor("w_out", [C, C], F32, kind="ExternalInput")
    out_d = nc.dram_tensor("out", [bpc, T, C], F32, kind="ExternalOutput")

    with tile.TileContext(nc) as tc:
        with tc.tile_pool(name="cst", bufs=1) as cst, \
             tc.tile_pool(name="sb", bufs=3) as sb, \
             tc.tile_pool(name="ps", bufs=1, space="PSUM") as ps:
            ident = cst.tile([128, 128], F32)
            make_identity(nc, ident[:])
            ident16 = cst.tile([128, 128], F16)
            nc.vector.tensor_copy(ident16[:], ident[:])
            # causal diag-block mask: mask16[p, l] = 1.0 if l >= p else 0.0
            mask16 = cst.tile([128, 128], F16)
            nc.vector.memset(mask16[:], 1.0)
            nc.gpsimd.affine_select(
                out=mask16[:], in_=mask16[:],
                compare_op=mybir.AluOpType.is_ge,
                fill=0.0, base=0, pattern=[[1, 128]],
                channel_multiplier=-1)

            # ---- one-time: transpose w_qkv -> wqT fp16 [2][96, 576] ----
            wq_sb = cst.tile([128, 5, 192], F32)
            wq_v = wqkv_d[0:512, :].rearrange("(n p) c -> p n c", p=128)
            nc.sync.dma_start(wq_sb[:, 0:4, :], wq_v)
            nc.sync.dma_start(wq_sb[0:64, 4, :], wqkv_d[512:576, :])
            wqT = []
            for cb in range(2):
                wt_ps = ps.tile([96, 512], F32, tag="s", bufs=2,
                                name=f"wt_ps{cb}")
                wt_ps2 = ps.tile([96, 128], F32, tag="a", bufs=2,
                                 name=f"wt_ps2{cb}")
                for ot in range(4):
                    nc.tensor.transpose(
                        wt_ps[:, ot * 128:ot * 128 + 128],
                        wq_sb[:, ot, cb * 96:cb * 96 + 96],
                        ident[:])
                nc.tensor.transpose(
                    wt_ps2[:, 0:64], wq_sb[0:64, 4, cb * 96:cb * 96 + 96],
                    ident[0:64, 0:64])
                w16 = cst.tile([96, 576], F16, name=f"wqT{cb}")
                nc.vector.tensor_copy(w16[:, 0:512], wt_ps[:])
                nc.vector.tensor_copy(w16[:, 512:576], wt_ps2[:, 0:64])
                wqT.append(w16)

            # ---- one-time: transpose w_out -> woT fp16 [96, 2, 192] ----
            wo_sb = cst.tile([128, 2, 192], F32)
            nc.sync.dma_start(wo_sb[:, 0, :], wout_d[0:128, :])
            nc.sync.dma_start(wo_sb[0:64, 1, :], wout_d[128:192, :])
            woT = cst.tile([96, 2, 192], F16, name="woT")
            for cb in range(2):
                wt_ps = ps.tile([96, 256], F32, tag="a", bufs=2,
                                name=f"wo_ps{cb}")
                for g, (rows, p0) in enumerate([(128, 0), (64, 128)]):
                    nc.tensor.transpose(
                        wt_ps[:, g * 128:g * 128 + rows],
                        wo_sb[0:rows, g, cb * 96:cb * 96 + 96],
                        ident[0:rows, 0:rows])
                nc.vector.tensor_copy(woT[:, cb, :], wt_ps[:, 0:192])

            x_v = x_d.rearrange("b (u p) c -> b p u c", p=128)
            o_v = out_d.rearrange("b (u p) c -> b p u c", p=128)

            def attn_tail(b, attn, v16):
                """Batch b's tail: attn@v, normalize, y^T, out-proj, store.
                Issued mid-body of batch b+1 so no engine's in-order queue
                parks early work of b+1 behind b's late dependencies."""
                # ---- attn @ [v | 1]: Y[tq, u, h, 33] (col 32 = rowsum) ----
                y_ps = ps.tile([128, 2, 256], F32, tag="y", bufs=2,
                               name=f"y{b}")
                for h in range(6):
                    p, e = h // 2, h % 2
                    apv, apm = attn[p]
                    ys0 = y_ps[:, 0, h * 33:h * 33 + 33]
                    ys1 = y_ps[:, 1, h * 33:h * 33 + 33]
                    nc.tensor.matmul(
                        ys0, apm[:, e * 128:e * 128 + 128],
                        v16[:, 0, h, :], start=True, stop=True,
                        skip_group_check=True)
                    nc.tensor.matmul(
                        ys1, apv[:, e * 128:e * 128 + 128],
                        v16[:, 0, h, :], start=True, stop=False,
                        skip_group_check=True)
                    nc.tensor.matmul(
                        ys1, apm[:, 256 + e * 128:256 + e * 128 + 128],
                        v16[:, 1, h, :], start=False, stop=True,
                        skip_group_check=True)

                # ---- normalize: y16[tq, u, h*32+d] = Y / rowsum ----
                y_hd = y_ps[:, :, 0:198].rearrange(
                    "p u (h d) -> p u h d", d=33)
                recip = sb.tile([128, 2, 6], F32, tag="rc", name=f"rc{b}")
                nc.vector.reciprocal(recip[:], y_hd[:, :, :, 32:33])
                y16 = sb.tile([128, 2, 192], F16, tag="y16", name=f"y16_{b}")
                yv = y_hd[:, :, :, 0:32]
                rb = recip[:].rearrange("p u (h o) -> p u h o", o=1)
                nc.vector.tensor_mul(
                    y16[:].rearrange("p u (h d) -> p u h d", h=6), yv,
                    rb.broadcast_to([128, 2, 6, 32]))

                # ---- y^T via PE transposes -> [96, (cb, u, 128)] ----
                yt_ps = ps.tile([96, 512], F16, tag="y", bufs=2,
                                name=f"yt{b}")
                for cb in range(2):
                    for u in range(2):
                        nc.tensor.transpose(
                            yt_ps[:, cb * 256 + u * 128:cb * 256 + u * 128 + 128],
                            y16[:, u, cb * 96:cb * 96 + 96], ident16[:])
                yT16 = sb.tile([96, 512], F16, tag="yT16", name=f"yT16_{b}")
                nc.vector.tensor_copy(yT16[:], yt_ps[:])

                # ---- out = y @ w_out.T ----
                o_ps = ps.tile([128, 2, 192], F32, tag="y", bufs=2,
                               name=f"o{b}")
                for u in range(2):
                    for cb in range(2):
                        nc.tensor.matmul(
                            o_ps[:, u, :],
                            yT16[:, cb * 256 + u * 128:cb * 256 + u * 128 + 128],
                            woT[:, cb, :],
                            start=(cb == 0), stop=(cb == 1))
                o_sb = sb.tile([128, 2, 192], F32, tag="ob", name=f"ob{b}")
                nc.vector.tensor_copy(o_sb[:], o_ps[:])
                nc.sync.dma_start(o_v[b], o_sb[:])

            rep_cm = tc.For_i(0, repeat) if repeat else contextlib.nullcontext()
            with rep_cm:
              # x-load prefetch: issue batch b+1's load before batch b's
              # out-store so SP's in-order queue never parks x behind it
              x_next = sb.tile([128, 2, 192], F32, tag="x", bufs=3, name="x0")
              nc.sync.dma_start(x_next[:], x_v[0])
              prev = None
              for b in range(bpc):
                x_sb = x_next
                if b + 1 < bpc:
                    x_next = sb.tile([128, 2, 192], F32, tag="x", bufs=3,
                                     name=f"x{b + 1}")
                    nc.sync.dma_start(x_next[:], x_v[b + 1])
                xt_ps = ps.tile([96, 512], F32, tag="a", bufs=2, name=f"xt{b}")
                for u in range(2):
                    for cb in range(2):
                        nc.tensor.transpose(
                            xt_ps[:, cb * 256 + u * 128:cb * 256 + u * 128 + 128],
                            x_sb[:, u, cb * 96:cb * 96 + 96], ident[:])
                xt16 = sb.tile([96, 512], F16, tag="xt16", name=f"xt16_{b}")
                nc.scalar.copy(xt16[:], xt_ps[:])

                # ---- q^T,k^T [o,t]: 3 o-tiles x 2 c-chunks ----
                # o-tiles 0,1 in a dedicated single bank; o-tile 2 shares
                # the xt tag, so next batch's qk waits only on the qk16 copy
                qk_a = ps.tile([128, 512], F32, tag="qk", bufs=2,
                               name=f"qka{b}")
                qk_b = ps.tile([128, 256], F32, tag="a", bufs=2,
                               name=f"qkb{b}")
                for ot in range(3):
                    dst = qk_a[:, ot * 256:ot * 256 + 256] if ot < 2 \
                        else qk_b[:]
                    for cb in range(2):
                        nc.tensor.matmul(
                            dst, wqT[cb][:, ot * 128:ot * 128 + 128],
                            xt16[:, cb * 256:cb * 256 + 256],
                            start=(cb == 0), stop=(cb == 1))
                # cast fp16 + DMA partition-quadrant shuffle -> [32, 3072]
                qk16 = sb.tile([128, 768], F16, tag="qk16", name=f"qk16_{b}")
                nc.vector.tensor_copy(qk16[:, 0:512], qk_a[:])
                nc.vector.tensor_copy(qk16[:, 512:768], qk_b[:])
                qkT32 = sb.tile([32, 3072], F16, tag="qkT32", name=f"qkT32_{b}")
                if merged_shuffle:
                    src = qk16[:].rearrange("(g p) c -> p g c", g=4)
                    dst = qkT32[:].rearrange("p (g c) -> p g c", g=4)
                    nc.sync.dma_start(dst, src)
                else:
                    for g in range(4):
                        nc.sync.dma_start(qkT32[0:32, g * 768:(g + 1) * 768],
                                          qk16[32 * g:32 * g + 32, :])

                # ---- v [t, o_v]: 2 t-tiles x 2 c-chunks ----
                v_ps = ps.tile([128, 384], F32, tag="y", bufs=2, name=f"v{b}")
                for u in range(2):
                    for cb in range(2):
                        nc.tensor.matmul(
                            v_ps[:, u * 192:u * 192 + 192],
                            xt16[:, cb * 256 + u * 128:cb * 256 + u * 128 + 128],
                            wqT[cb][:, 384:576],
                            start=(cb == 0), stop=(cb == 1))
                # v16 [tk-local, u, h, 33] with ones in col 32 (rowsum trick)
                v16 = sb.tile([128, 2, 6, 33], F16, tag="v16", name=f"v16_{b}")
                nc.vector.memset(v16[:, :, :, 32:33], 1.0)
                nc.vector.tensor_copy(
                    v16[:, :, :, 0:32],
                    v_ps[:].rearrange("p (u h d) -> p u h d", u=2, h=6))

                # ---- software-pipelined tail of the previous batch ----
                # (overlaps the shuffle-DMA latency before this batch's
                # score matmuls can start)
                if prev is not None:
                    attn_tail(*prev)

                # ---- scores S^T per head-pair ----
                # spD [128, 512] = all 4 causal-diagonal blocks, e = h % 2:
                #   {e*128: (tk0,q0)}  {256+e*128: (tk1,q1)}
                # spV [128, 256] = the 2 fully-valid (tk0,q1) blocks
                attn = []   # (apv, apm) per pair
                mb = mask16[:].rearrange("p (i l) -> p i l", i=1)
                for p in range(3):
                    spD = ps.tile([128, 512], F32, tag="s", bufs=2,
                                  name=f"sd{p}_{b}")
                    spV = ps.tile([128, 256], F32, tag="s", bufs=2,
                                  name=f"sv{p}_{b}")
                    for e in range(2):
                        h = 2 * p + e
                        qc, kc = QCOL[h], KCOL[h]
                        nc.tensor.matmul(
                            spD[:, e * 128:e * 128 + 128],
                            qkT32[0:32, kc:kc + 128],
                            qkT32[0:32, qc:qc + 128],
                            start=True, stop=True, tile_position=(0, 0))
                        nc.tensor.matmul(
                            spV[:, e * 128:e * 128 + 128],
                            qkT32[0:32, kc:kc + 128],
                            qkT32[0:32, qc + 128:qc + 256],
                            start=True, stop=True, tile_position=(0, 0))
                        nc.tensor.matmul(
                            spD[:, 256 + e * 128:256 + e * 128 + 128],
                            qkT32[0:32, kc + 128:kc + 256],
                            qkT32[0:32, qc + 128:qc + 256],
                            start=True, stop=True, tile_position=(0, 0))
                    apd = sb.tile([128, 512], F16, tag=f"atd{p}",
                                  name=f"atd{p}_{b}")
                    nc.scalar.activation(apd[:], spD[:], Exp,
                                         scale=float(SCALE))
                    apv = sb.tile([128, 256], F16, tag=f"atv{p}",
                                  name=f"atv{p}_{b}")
                    nc.scalar.activation(apv[:], spV[:], Exp,
                                         scale=float(SCALE))
                    apm = sb.tile([128, 512], F16, tag=f"am{p}",
                                  name=f"am{p}_{b}")
                    # mask all 4 diag blocks in one multiply (Pool, SBUF)
                    nc.gpsimd.tensor_mul(
                        apm[:].rearrange("p (i l) -> p i l", l=128),
                        apd[:].rearrange("p (i l) -> p i l", l=128),
                        mb.broadcast_to([128, 4, 128]))
                    attn.append((apv, apm))

                prev = (b, attn, v16)
              attn_tail(*prev)

    nc.compile()
    return nc


def _get_nc():
    if "nc" not in _CACHE:
        _CACHE["nc"] = _build()
    return _CACHE["nc"]


def kernel(x: np.ndarray, w_qkv: np.ndarray, w_out: np.ndarray) -> np.ndarray:
    from concourse.bass_utils import run_bass_kernel_spmd

    nc = _get_nc()
    x = np.ascontiguousarray(np.asarray(x, dtype=np.float32))
    w_qkv = np.ascontiguousarray(np.asarray(w_qkv, dtype=np.float32))
    w_out = np.ascontiguousarray(np.asarray(w_out, dtype=np.float32))
    in_maps = [
        {"x": x[i * BPC:(i + 1) * BPC], "w_qkv": w_qkv, "w_out": w_out}
        for i in range(NCORES)
    ]
    res = run_bass_kernel_spmd(nc, in_maps, core_ids=list(range(NCORES)))
    out = np.concatenate([r["out"] for r in res.results], axis=0)
    return out.astype(np.float32)
